# revision 32
# baseline (speedup 1.0000x reference)
"""Trainium2 Bass kernel for nn_Encoder (dense transformer block), 8 NeuronCores.

Strategy (single chip, 8 cores):
  Phase A (head-parallel): core c computes attention for heads {2c, 2c+1}.
    All activations are kept "transposed" (feature dim on SBUF partitions) so
    every matmul consumes naturally-laid-out operands and no fp32 transposes
    are ever needed on device; the host pre-transposes X and all weights.
    softmax(relu(s)) is computed as p = max(exp(s/8), 1) (exp is monotonic),
    and the row sums come for free as a 65th column of the p @ [V | 1] matmul.
  Host gathers per-head ctx blocks (2 MiB/core) between launches.
  Phase B (row-parallel): core c takes 512 of the 4096 token rows:
    ctx @ Wo.T (+X residual), LN1, FFN (ReLU), LN2. LayerNorm reductions run
    over the partition dim via tiny ones-vector matmuls on the PE.

kernel() is self-contained: it compiles both phase programs on first call
(cached in module globals) and runs them via run_bass_kernel_spmd.
"""

import os
import sys

for _p in ("/opt/trn_rl_repo",):
    if _p not in sys.path:
        sys.path.insert(0, _p)

# The Bass SPMD path executes through jax/PJRT on the axon platform; make
# sure a caller-pinned JAX_PLATFORMS=cpu doesn't hide the NeuronCores.
_jp = os.environ.get("JAX_PLATFORMS")
if _jp is not None and "axon" not in _jp:
    os.environ["JAX_PLATFORMS"] = "axon," + _jp

import numpy as np

import concourse.bass as bass
import concourse.mybir as mybir
import concourse.tile as tile
from concourse import bacc
from concourse.bass_utils import run_bass_kernel_spmd

F32 = mybir.dt.float32
F32R = mybir.dt.float32r
AF = mybir.ActivationFunctionType
OP = mybir.AluOpType


def _mm(nc, out, lhsT, rhs, **kw):
    # fp32r: 1-pass FP22 matmul (4x faster than 4-pass true-fp32 for N>=256)
    nc.tensor.matmul(out, lhsT.bitcast(F32R), rhs.bitcast(F32R), **kw)

N_CORES = 8
B, S, D, H, DH, FF = 2, 2048, 1024, 16, 64, 4096
N = B * S            # 4096 token rows
P = 128
QC = N // N_CORES    # 512 rows per core in phase B
HPC = H // N_CORES   # 2 heads per core in phase A
EPS = 1e-5

_CACHE = {}


# --------------------------------------------------------------------------
# Phase A: per-core head-parallel attention.
# Inputs (per core):
#   xt  [D, N]     X^T (full, replicated)
#   wqt [D, 128]   Wq^T columns for this core's two heads
#   wkt [D, 128]   Wk^T columns
#   wvt [D, 128]   Wo^T columns (value projection uses W_o in this model)
# Output:
#   ctx_t [128, N] softmax(relu(qk/8)) @ v, transposed; rows = the two heads'
#                  64-dim blocks stacked, cols = (b, s) token index.
# --------------------------------------------------------------------------
def _build_phase_a():
    nc = bacc.Bacc("TRN2", target_bir_lowering=False, debug=False,
                   num_devices=N_CORES)
    xt = nc.dram_tensor("xt", [D, N], F32R, kind="ExternalInput")
    wqt = nc.dram_tensor("wqt", [D, P], F32R, kind="ExternalInput")
    wkt = nc.dram_tensor("wkt", [D, P], F32R, kind="ExternalInput")
    wvt = nc.dram_tensor("wvt", [D, P], F32R, kind="ExternalInput")
    idm = nc.dram_tensor("idm", [P, DH], F32R, kind="ExternalInput")
    ctx_t = nc.dram_tensor("ctx_t", [P, N], F32, kind="ExternalOutput")

    KD = D // P        # 8 contraction chunks over D
    NQ = N // 512      # 8 qi chunks of 512 over all tokens
    KI = S // P        # 16 ki chunks of 128 per batch

    with tile.TileContext(nc) as tc:
        with tc.tile_pool(name="persist", bufs=1) as persist:
            # Persistent SBUF, split per batch so batch-1 projection writes
            # never serialize against batch-0 attention reads: projected Q^T,
            # K^T, V^T (1 MiB each per batch) and V' (natural layout per
            # ki-chunk: [v_h0(64) | 1 | v_h1(64) | 1]).
            qt_sb = [persist.tile([P, S], F32R, name=f"qt{b_}") for b_ in range(B)]
            kt_sb = [persist.tile([P, S], F32R, name=f"kt{b_}") for b_ in range(B)]
            vt_sb = [persist.tile([P, S], F32R, name=f"vt{b_}") for b_ in range(B)]
            vp_sb = [persist.tile([P, KI, 2 * (DH + 1)], F32R, name=f"vp{b_}")
                     for b_ in range(B)]
            wq_sb = persist.tile([P, KD, P], F32R)
            wk_sb = persist.tile([P, KD, P], F32R)
            wv_sb = persist.tile([P, KD, P], F32R)
            id_sb = persist.tile([P, DH], F32R)

            nc.sync.dma_start(wq_sb[:], wqt.ap().rearrange("(kc p) m -> p kc m", p=P))
            nc.sync.dma_start(wk_sb[:], wkt.ap().rearrange("(kc p) m -> p kc m", p=P))
            nc.sync.dma_start(wv_sb[:], wvt.ap().rearrange("(kc p) m -> p kc m", p=P))
            nc.sync.dma_start(id_sb[:], idm.ap())
            for b_ in range(B):
                # ones columns of V' (columns DH and 2*DH+1)
                nc.vector.memset(vp_sb[b_][:, :, DH:DH + 1].bitcast(F32), 1.0)
                nc.vector.memset(
                    vp_sb[b_][:, :, 2 * DH + 1:2 * DH + 2].bitcast(F32), 1.0)

            # ---------------- fused projections + attention ----------------
            # Projections run in t-layout (N=512 keeps fp32r at 1 cyc/row); V
            # is PE-transposed into natural layout for the ctx matmul. Batch
            # 0's projections form the prologue; batch 1's are interleaved
            # into batch 0's attention chunks to fill the PE slack while the
            # ScalarE exp pass (the bottleneck) runs. The attention itself is
            # software-pipelined at ki-chunk granularity: chunk i+1's score
            # matmuls interleave with chunk i's ctx matmuls.
            with (
                tc.tile_pool(name="xa", bufs=9) as xpool,
                tc.tile_pool(name="accp", bufs=2, space="PSUM") as accp,
                tc.tile_pool(name="slabp", bufs=19) as slabp,
                tc.tile_pool(name="smallp", bufs=2) as smallp,
                tc.tile_pool(name="coutp", bufs=2) as coutp,
                tc.tile_pool(name="pss", bufs=2, space="PSUM") as pss,
                tc.tile_pool(name="psc", bufs=1, space="PSUM") as psc,
            ):

                def proj_chunk(b_, o):
                    """Project one 512-token slice of batch b_ into qt/kt/vt.

                    Three sequential PSUM accumulation chains over a shared
                    single-slot pool tag keep the PSUM footprint at 2 banks.
                    """
                    tiles = []
                    for kc in range(KD):
                        xt_tile = xpool.tile([P, 512], F32R, name="xt_tile")
                        nc.sync.dma_start(
                            xt_tile[:],
                            xt[kc * P:(kc + 1) * P,
                               b_ * S + o * 512:b_ * S + (o + 1) * 512])
                        tiles.append(xt_tile)
                    for w_sb, dst in ((wq_sb, qt_sb[b_]), (wk_sb, kt_sb[b_]),
                                      (wv_sb, vt_sb[b_])):
                        acc = accp.tile([P, 512], F32, name="acc_ps")
                        for kc in range(KD):
                            _mm(nc, acc[:], w_sb[:, kc], tiles[kc][:],
                                start=(kc == 0), stop=(kc == KD - 1))
                        nc.vector.tensor_copy(
                            dst[:, o * 512:(o + 1) * 512], acc[:])

                def transp_chunk(b_, kc2):
                    """PE-transpose one [64,128] V^T block per head into V'."""
                    for hh in range(2):
                        tp = accp.tile([P, DH], F32R, name="acc_ps")
                        nc.tensor.transpose(
                            tp[:, :DH],
                            vt_sb[b_][hh * DH:(hh + 1) * DH,
                                      kc2 * P:(kc2 + 1) * P],
                            id_sb[hh * DH:(hh + 1) * DH, :])
                        nc.vector.tensor_copy(
                            vp_sb[b_][:, kc2,
                                      hh * (DH + 1):hh * (DH + 1) + DH],
                            tp[:, :DH])
                chunks = [(b_, o) for b_ in range(B) for o in range(S // 512)]
                state = {}

                def emit_scores(idx, kc):
                    b_, o = chunks[idx]
                    qs = slice(o * 512, (o + 1) * 512)
                    ks = slice(kc * P, (kc + 1) * P)
                    s_ps = pss.tile([P, 1024], F32, name="s_ps")
                    _mm(nc, s_ps[:, 0:512], kt_sb[b_][0:DH, ks],
                        qt_sb[b_][0:DH, qs], start=True, stop=True)
                    _mm(nc, s_ps[:, 512:1024], kt_sb[b_][DH:2 * DH, ks],
                        qt_sb[b_][DH:2 * DH, qs], start=True, stop=True)
                    slab = slabp.tile([P, 1024], F32R, name="slab")
                    nc.scalar.activation(slab[:], s_ps[:], AF.Exp, scale=0.125)
                    nc.vector.tensor_scalar_max(slab[:], slab[:], 1.0)
                    state[idx]["slabs"].append(slab)

                def emit_ctx(idx, kc):
                    b_, o = chunks[idx]
                    st_, sp_ = kc == 0, kc == KI - 1
                    c0, c1 = state[idx]["c0"], state[idx]["c1"]
                    slab = state[idx]["slabs"][kc]
                    _mm(nc, c0[:], vp_sb[b_][:, kc, 0:DH + 1], slab[:, 0:512],
                        start=st_, stop=sp_)
                    _mm(nc, c1[:], vp_sb[b_][:, kc, DH + 1:2 * DH + 2],
                        slab[:, 512:1024], start=st_, stop=sp_)

                def emit_normalize(idx):
                    b_, o = chunks[idx]
                    qs = slice(b_ * S + o * 512, b_ * S + (o + 1) * 512)
                    c0, c1 = state[idx]["c0"], state[idx]["c1"]
                    inv0 = smallp.tile([1, 512], F32, name="inv0")
                    inv1 = smallp.tile([1, 512], F32, name="inv1")
                    nc.vector.reciprocal(inv0[:], c0[DH:DH + 1, :])
                    nc.vector.reciprocal(inv1[:], c1[DH:DH + 1, :])
                    inv0b = smallp.tile([DH, 512], F32, name="inv0b")
                    inv1b = smallp.tile([DH, 512], F32, name="inv1b")
                    nc.gpsimd.partition_broadcast(inv0b[:], inv0[:])
                    nc.gpsimd.partition_broadcast(inv1b[:], inv1[:])
                    cout0 = coutp.tile([DH, 512], F32, name="cout0")
                    cout1 = coutp.tile([DH, 512], F32, name="cout1")
                    nc.vector.tensor_mul(cout0[:], c0[0:DH, :], inv0b[:])
                    nc.vector.tensor_mul(cout1[:], c1[0:DH, :], inv1b[:])
                    nc.sync.dma_start(ctx_t[0:DH, qs], cout0[:])
                    nc.sync.dma_start(ctx_t[DH:2 * DH, qs], cout1[:])
                    del state[idx]

                # prologue: batch 0 only
                for o in range(S // 512):
                    proj_chunk(0, o)
                for kc2 in range(KI):
                    transp_chunk(0, kc2)
                # attention, with batch-1 projections/transposes interleaved
                # into batch-0's chunks (idx 0..3)
                for idx in range(len(chunks)):
                    state[idx] = {
                        "c0": psc.tile([DH + 1, 512], F32, name="c0"),
                        "c1": psc.tile([DH + 1, 512], F32, name="c1"),
                        "slabs": [],
                    }
                    for kc in range(KI):
                        emit_scores(idx, kc)
                        if idx > 0:
                            emit_ctx(idx - 1, kc)
                    if idx < S // 512:
                        proj_chunk(1, idx)
                        for t in range(KI // (S // 512)):
                            transp_chunk(1, idx * (KI // (S // 512)) + t)
                    if idx > 0:
                        emit_normalize(idx - 1)
                last = len(chunks) - 1
                for kc in range(KI):
                    emit_ctx(last, kc)
                emit_normalize(last)
    nc.compile()
    return nc


# --------------------------------------------------------------------------
# Phase B: per-core row-parallel Wo-proj + AddNorm1 + FFN + AddNorm2.
# Inputs (per core, qi = this core's 512 token rows):
#   ct  [D, QC]    ctx^T slice
#   xts [D, QC]    X^T slice (residual 1)
#   wot [D, D]     Wo^T
#   w1t [D, FF]    W1^T
#   w2t [FF, D]    W2^T
#   g1,be1,g2,be2 [P, D//P]  ln params, feature-on-partition layout
#   b1t [P, FF//P], b2t [P, D//P]
# Output: out_t [D, QC]
# --------------------------------------------------------------------------
def _build_phase_b():
    nc = bacc.Bacc("TRN2", target_bir_lowering=False, debug=False,
                   num_devices=N_CORES)
    ct = nc.dram_tensor("ct", [D, QC], F32R, kind="ExternalInput")
    xts = nc.dram_tensor("xts", [D, QC], F32, kind="ExternalInput")
    wot = nc.dram_tensor("wot", [D, D], F32R, kind="ExternalInput")
    w1t = nc.dram_tensor("w1t", [D, FF], F32R, kind="ExternalInput")
    w2t = nc.dram_tensor("w2t", [FF, D], F32R, kind="ExternalInput")
    g1 = nc.dram_tensor("g1", [P, D // P], F32, kind="ExternalInput")
    be1 = nc.dram_tensor("be1", [P, D // P], F32, kind="ExternalInput")
    g2 = nc.dram_tensor("g2", [P, D // P], F32, kind="ExternalInput")
    be2 = nc.dram_tensor("be2", [P, D // P], F32, kind="ExternalInput")
    b1t = nc.dram_tensor("b1t", [P, FF // P], F32, kind="ExternalInput")
    b2t = nc.dram_tensor("b2t", [P, D // P], F32, kind="ExternalInput")
    out_t = nc.dram_tensor("out_t", [D, QC], F32, kind="ExternalOutput")

    KD = D // P     # 8
    KF = FF // P    # 32

    def layernorm(nc, tc, pools, y_sb, g_sb, be_sb, z_sb, ones, tag):
        """t-layout layernorm: y_sb [P, KD, 512] -> z_sb (may alias layout)."""
        smallp, sqp, bcp = pools
        import contextlib
        ctx = contextlib.ExitStack()
        psst = ctx.enter_context(
            tc.tile_pool(name=f"psst_{tag}", bufs=1, space="PSUM"))
        st_ps = psst.tile([1, 1024], F32, name="st_ps")
        for kc in range(KD):
            _mm(nc, st_ps[:, 0:512], ones[:], y_sb[:, kc],
                             start=(kc == 0), stop=(kc == KD - 1))
        for kc in range(KD):
            sq = sqp.tile([P, 512], F32R, name="sq")
            nc.vector.tensor_mul(sq[:], y_sb[:, kc], y_sb[:, kc])
            _mm(nc, st_ps[:, 512:1024], ones[:], sq[:],
                             start=(kc == 0), stop=(kc == KD - 1))
        stats = smallp.tile([1, 1024], F32, name="stats")
        nc.vector.tensor_scalar(out=stats[:], in0=st_ps[:], scalar1=1.0 / D,
                                scalar2=None, op0=OP.mult)
        mean = stats[:, 0:512]
        ex2 = stats[:, 512:1024]
        msq = smallp.tile([1, 512], F32, name="msq")
        nc.vector.tensor_mul(msq[:], mean, mean)
        var = smallp.tile([1, 512], F32, name="var")
        nc.vector.tensor_sub(var[:], ex2, msq[:])
        nc.vector.tensor_scalar_add(var[:], var[:], EPS)
        std = smallp.tile([1, 512], F32, name="std")
        nc.scalar.activation(std[:], var[:], AF.Sqrt)
        rstd = smallp.tile([1, 512], F32, name="rstd")
        nc.vector.reciprocal(rstd[:], std[:])
        ms = smallp.tile([1, 512], F32, name="ms")
        nc.vector.tensor_mul(ms[:], mean, rstd[:])
        rstd_b = bcp.tile([P, 512], F32, name="rstd_b")
        ms_b = bcp.tile([P, 512], F32, name="ms_b")
        nc.gpsimd.partition_broadcast(rstd_b[:], rstd[:])
        nc.gpsimd.partition_broadcast(ms_b[:], ms[:])
        for kc in range(KD):
            t = sqp.tile([P, 512], F32, name="t_ln")
            nc.vector.tensor_mul(t[:], y_sb[:, kc], rstd_b[:])
            nc.vector.tensor_sub(t[:], t[:], ms_b[:])
            nc.vector.tensor_scalar(out=z_sb[:, kc], in0=t[:],
                                    scalar1=g_sb[:, kc:kc + 1],
                                    scalar2=be_sb[:, kc:kc + 1],
                                    op0=OP.mult, op1=OP.add)
        ctx.close()

    with tile.TileContext(nc) as tc:
        with (
            tc.tile_pool(name="persist", bufs=1) as persist,
            tc.tile_pool(name="wp", bufs=6) as wp,
            tc.tile_pool(name="sqp", bufs=3) as sqp,
            tc.tile_pool(name="smallp", bufs=1) as smallp,
            tc.tile_pool(name="bcp", bufs=2) as bcp,
        ):
            ct_sb = persist.tile([P, KD, QC], F32R)
            xts_sb = persist.tile([P, KD, QC], F32)
            y1_sb = persist.tile([P, KD, QC], F32R)
            z1_sb = persist.tile([P, KD, QC], F32R)
            h_sb = persist.tile([P, KF, QC], F32R)
            # y2 reuses y1's slot (y1 dead after LN1); z2 reuses ct's (dead
            # after B1). Tag sharing makes Tile serialize via WAR edges.
            y2_sb = persist.tile([P, KD, QC], F32R, tag="y1_sb")
            z2_sb = persist.tile([P, KD, QC], F32, tag="ct_sb")
            g1_sb = persist.tile([P, KD], F32)
            be1_sb = persist.tile([P, KD], F32)
            g2_sb = persist.tile([P, KD], F32)
            be2_sb = persist.tile([P, KD], F32)
            b1t_sb = persist.tile([P, KF], F32)
            b2t_sb = persist.tile([P, KD], F32)
            ones = persist.tile([P, 1], F32R)

            nc.sync.dma_start(ct_sb[:], ct.ap().rearrange("(kc p) q -> p kc q", p=P))
            nc.sync.dma_start(xts_sb[:], xts.ap().rearrange("(kc p) q -> p kc q", p=P))
            for t_sb, t_dr in ((g1_sb, g1), (be1_sb, be1), (g2_sb, g2),
                               (be2_sb, be2), (b1t_sb, b1t), (b2t_sb, b2t)):
                nc.sync.dma_start(t_sb[:], t_dr.ap())
            nc.vector.memset(ones[:].bitcast(F32), 1.0)

            # ---- B1: att_out = Wo @ ct (+ X residual) ----
            with tc.tile_pool(name="psa", bufs=1, space="PSUM") as psa:
                for mg in range(2):
                    a_ps = [psa.tile([P, 512], F32, name=f"mm_ps{i}")
                            for i in range(4)]
                    for kc in range(KD):
                        w_tile = wp.tile([P, 512], F32R, name="wo_tile")
                        nc.sync.dma_start(
                            w_tile[:],
                            wot[kc * P:(kc + 1) * P, mg * 512:(mg + 1) * 512])
                        for i in range(4):
                            _mm(nc, a_ps[i][:],
                                w_tile[:, i * P:(i + 1) * P], ct_sb[:, kc],
                                start=(kc == 0), stop=(kc == KD - 1))
                    for i in range(4):
                        m = mg * 4 + i
                        nc.vector.tensor_add(y1_sb[:, m], a_ps[i][:], xts_sb[:, m])

                # ---- LN1 ----
                layernorm(nc, tc, (smallp, sqp, bcp), y1_sb, g1_sb, be1_sb,
                          z1_sb, ones, "ln1")

            # ---- FFN1 + FFN2: the first output half of FFN2 (mg0) is
            # interleaved into the FFN1 loop so W2 @ h starts consuming h
            # chunks as soon as they exist; tile sizes stay [128,512] so the
            # DMA instruction count is unchanged. 8 PSUM banks: 4 h + 4 f. ----
            with tc.tile_pool(name="psa2", bufs=1, space="PSUM") as psa2:
                f_ps = [psa2.tile([P, 512], F32, name=f"f_ps{i}")
                        for i in range(4)]
                for fg in range(KF // 4):
                    h_ps = [psa2.tile([P, 512], F32, name=f"h_ps{i}")
                            for i in range(4)]
                    for kc in range(KD):
                        w_tile = wp.tile([P, 512], F32R, name="w1_tile")
                        nc.sync.dma_start(
                            w_tile[:],
                            w1t[kc * P:(kc + 1) * P, fg * 512:(fg + 1) * 512])
                        for i in range(4):
                            _mm(nc, h_ps[i][:],
                                w_tile[:, i * P:(i + 1) * P], z1_sb[:, kc],
                                start=(kc == 0), stop=(kc == KD - 1))
                    for i in range(4):
                        fm = fg * 4 + i
                        nc.scalar.activation(h_sb[:, fm], h_ps[i][:], AF.Relu,
                                             bias=b1t_sb[:, fm:fm + 1])
                    for i in range(4):
                        fk = fg * 4 + i
                        w_tile = wp.tile([P, 512], F32R, name="w2_tile")
                        nc.sync.dma_start(
                            w_tile[:], w2t[fk * P:(fk + 1) * P, 0:512])
                        for j in range(4):
                            _mm(nc, f_ps[j][:],
                                w_tile[:, j * P:(j + 1) * P], h_sb[:, fk],
                                start=(fk == 0), stop=(fk == KF - 1))
                for j in range(4):
                    nc.vector.scalar_tensor_tensor(
                        out=y2_sb[:, j], in0=f_ps[j][:],
                        scalar=b2t_sb[:, j:j + 1], in1=z1_sb[:, j],
                        op0=OP.add, op1=OP.add)

                f_ps2 = [psa2.tile([P, 512], F32, name=f"f_ps{i}")
                         for i in range(4)]
                for fk in range(KF):
                    w_tile = wp.tile([P, 512], F32R, name="w2_tile")
                    nc.sync.dma_start(
                        w_tile[:], w2t[fk * P:(fk + 1) * P, 512:1024])
                    for j in range(4):
                        _mm(nc, f_ps2[j][:],
                            w_tile[:, j * P:(j + 1) * P], h_sb[:, fk],
                            start=(fk == 0), stop=(fk == KF - 1))
                for j in range(4):
                    m = 4 + j
                    nc.vector.scalar_tensor_tensor(
                        out=y2_sb[:, m], in0=f_ps2[j][:],
                        scalar=b2t_sb[:, m:m + 1], in1=z1_sb[:, m],
                        op0=OP.add, op1=OP.add)

            # ---- LN2 ----
            layernorm(nc, tc, (smallp, sqp, bcp), y2_sb, g2_sb, be2_sb,
                      z2_sb, ones, "ln2")

            for kc in range(KD):
                nc.sync.dma_start(out_t[kc * P:(kc + 1) * P, :], z2_sb[:, kc])
    nc.compile()
    return nc


def _get(name, builder):
    if name not in _CACHE:
        _CACHE[name] = builder()
    return _CACHE[name]


def _prep_inputs(X, Wq, Wk, Wo, ln1_g, ln1_b, ln2_g, ln2_b, W1, b1, W2, b2):
    """Host-side sharding/layout. Returns (in_maps_a, in_maps_b_builder, Xt)."""
    f = lambda a: np.ascontiguousarray(np.asarray(a, dtype=np.float32))
    Xt = f(np.asarray(X, np.float32).reshape(N, D).T)        # [D, N]
    WqT, WkT, WoT = f(np.asarray(Wq).T), f(np.asarray(Wk).T), f(np.asarray(Wo).T)
    W1T, W2T = f(np.asarray(W1).T), f(np.asarray(W2).T)      # [D,FF], [FF,D]
    vecP = lambda v, k: f(np.asarray(v).reshape(k, P).T)     # [P, k]
    g1v, be1v = vecP(ln1_g, D // P), vecP(ln1_b, D // P)
    g2v, be2v = vecP(ln2_g, D // P), vecP(ln2_b, D // P)
    b1v, b2v = vecP(b1, FF // P), vecP(b2, D // P)

    idm = np.tile(np.eye(DH, dtype=np.float32), (2, 1))   # [128, 64]
    in_maps_a = [
        {
            "xt": Xt,
            "idm": idm,
            "wqt": f(WqT[:, c * P:(c + 1) * P]),
            "wkt": f(WkT[:, c * P:(c + 1) * P]),
            "wvt": f(WoT[:, c * P:(c + 1) * P]),
        }
        for c in range(N_CORES)
    ]

    def in_maps_b(ct_full):
        return [
            {
                "ct": f(ct_full[:, c * QC:(c + 1) * QC]),
                "xts": f(Xt[:, c * QC:(c + 1) * QC]),
                "wot": WoT, "w1t": W1T, "w2t": W2T,
                "g1": g1v, "be1": be1v, "g2": g2v, "be2": be2v,
                "b1t": b1v, "b2t": b2v,
            }
            for c in range(N_CORES)
        ]

    return in_maps_a, in_maps_b


def kernel(X, Wq, Wk, Wo, ln1_g, ln1_b, ln2_g, ln2_b, W1, b1, W2, b2):
    in_maps_a, in_maps_b = _prep_inputs(
        X, Wq, Wk, Wo, ln1_g, ln1_b, ln2_g, ln2_b, W1, b1, W2, b2)

    nc_a = _get("a", _build_phase_a)
    res_a = run_bass_kernel_spmd(nc_a, in_maps_a, core_ids=list(range(N_CORES)))
    ct_full = np.concatenate(
        [res_a.results[c]["ctx_t"] for c in range(N_CORES)], axis=0)  # [D, N]

    nc_b = _get("b", _build_phase_b)
    res_b = run_bass_kernel_spmd(nc_b, in_maps_b(ct_full),
                                 core_ids=list(range(N_CORES)))
    out_t = np.concatenate(
        [res_b.results[c]["out_t"] for c in range(N_CORES)], axis=1)  # [D, N]
    return np.ascontiguousarray(out_t.T).reshape(B, S, D).astype(np.float32)


# revision 35
# speedup vs baseline: 1.0080x; 1.0080x over previous
"""Trainium2 Bass kernel for nn_Encoder (dense transformer block), 8 NeuronCores.

Strategy (single chip, 8 cores):
  Phase A (head-parallel): core c computes attention for heads {2c, 2c+1}.
    All activations are kept "transposed" (feature dim on SBUF partitions) so
    every matmul consumes naturally-laid-out operands and no fp32 transposes
    are ever needed on device; the host pre-transposes X and all weights.
    softmax(relu(s)) is computed as p = max(exp(s/8), 1) (exp is monotonic),
    and the row sums come for free as a 65th column of the p @ [V | 1] matmul.
  Host gathers per-head ctx blocks (2 MiB/core) between launches.
  Phase B (row-parallel): core c takes 512 of the 4096 token rows:
    ctx @ Wo.T (+X residual), LN1, FFN (ReLU), LN2. LayerNorm reductions run
    over the partition dim via tiny ones-vector matmuls on the PE.

kernel() is self-contained: it compiles both phase programs on first call
(cached in module globals) and runs them via run_bass_kernel_spmd.
"""

import os
import sys

for _p in ("/opt/trn_rl_repo",):
    if _p not in sys.path:
        sys.path.insert(0, _p)

# The Bass SPMD path executes through jax/PJRT on the axon platform; make
# sure a caller-pinned JAX_PLATFORMS=cpu doesn't hide the NeuronCores.
_jp = os.environ.get("JAX_PLATFORMS")
if _jp is not None and "axon" not in _jp:
    os.environ["JAX_PLATFORMS"] = "axon," + _jp

import numpy as np

import concourse.bass as bass
import concourse.mybir as mybir
import concourse.tile as tile
from concourse import bacc
from concourse.bass_utils import run_bass_kernel_spmd

F32 = mybir.dt.float32
F32R = mybir.dt.float32r
AF = mybir.ActivationFunctionType
OP = mybir.AluOpType


def _mm(nc, out, lhsT, rhs, **kw):
    # fp32r: 1-pass FP22 matmul (4x faster than 4-pass true-fp32 for N>=256)
    nc.tensor.matmul(out, lhsT.bitcast(F32R), rhs.bitcast(F32R), **kw)

N_CORES = 8
B, S, D, H, DH, FF = 2, 2048, 1024, 16, 64, 4096
N = B * S            # 4096 token rows
P = 128
QC = N // N_CORES    # 512 rows per core in phase B
HPC = H // N_CORES   # 2 heads per core in phase A
EPS = 1e-5

_CACHE = {}


# --------------------------------------------------------------------------
# Phase A: per-core head-parallel attention.
# Inputs (per core):
#   xt  [D, N]     X^T (full, replicated)
#   wqt [D, 128]   Wq^T columns for this core's two heads
#   wkt [D, 128]   Wk^T columns
#   wvt [D, 128]   Wo^T columns (value projection uses W_o in this model)
# Output:
#   ctx_t [128, N] softmax(relu(qk/8)) @ v, transposed; rows = the two heads'
#                  64-dim blocks stacked, cols = (b, s) token index.
# --------------------------------------------------------------------------
def _build_phase_a():
    nc = bacc.Bacc("TRN2", target_bir_lowering=False, debug=False,
                   num_devices=N_CORES)
    xt = nc.dram_tensor("xt", [D, N], F32R, kind="ExternalInput")
    wqt = nc.dram_tensor("wqt", [D, P], F32R, kind="ExternalInput")
    wkt = nc.dram_tensor("wkt", [D, P], F32R, kind="ExternalInput")
    wvt = nc.dram_tensor("wvt", [D, P], F32R, kind="ExternalInput")
    idm = nc.dram_tensor("idm", [P, DH], F32R, kind="ExternalInput")
    ctx_t = nc.dram_tensor("ctx_t", [P, N], F32, kind="ExternalOutput")

    KD = D // P        # 8 contraction chunks over D
    NQ = N // 512      # 8 qi chunks of 512 over all tokens
    KI = S // P        # 16 ki chunks of 128 per batch

    with tile.TileContext(nc) as tc:
        with tc.tile_pool(name="persist", bufs=1) as persist:
            # Persistent SBUF, split per batch so batch-1 projection writes
            # never serialize against batch-0 attention reads: projected Q^T,
            # K^T, V^T (1 MiB each per batch) and V' (natural layout per
            # ki-chunk: [v_h0(64) | 1 | v_h1(64) | 1]).
            qt_sb = [persist.tile([P, S], F32R, name=f"qt{b_}") for b_ in range(B)]
            kt_sb = [persist.tile([P, S], F32R, name=f"kt{b_}") for b_ in range(B)]
            vt_sb = [persist.tile([P, S], F32R, name=f"vt{b_}") for b_ in range(B)]
            vp_sb = [persist.tile([P, KI, 2 * (DH + 1)], F32R, name=f"vp{b_}")
                     for b_ in range(B)]
            wq_sb = persist.tile([P, KD, P], F32R)
            wk_sb = persist.tile([P, KD, P], F32R)
            wv_sb = persist.tile([P, KD, P], F32R)
            id_sb = persist.tile([P, DH], F32R)

            nc.sync.dma_start(wq_sb[:], wqt.ap().rearrange("(kc p) m -> p kc m", p=P))
            nc.sync.dma_start(wk_sb[:], wkt.ap().rearrange("(kc p) m -> p kc m", p=P))
            nc.sync.dma_start(wv_sb[:], wvt.ap().rearrange("(kc p) m -> p kc m", p=P))
            nc.sync.dma_start(id_sb[:], idm.ap())
            for b_ in range(B):
                # ones columns of V' (columns DH and 2*DH+1)
                nc.vector.memset(vp_sb[b_][:, :, DH:DH + 1].bitcast(F32), 1.0)
                nc.vector.memset(
                    vp_sb[b_][:, :, 2 * DH + 1:2 * DH + 2].bitcast(F32), 1.0)

            # ---------------- fused projections + attention ----------------
            # Projections run in t-layout (N=512 keeps fp32r at 1 cyc/row); V
            # is PE-transposed into natural layout for the ctx matmul. Batch
            # 0's projections form the prologue; batch 1's are interleaved
            # into batch 0's attention chunks to fill the PE slack while the
            # ScalarE exp pass (the bottleneck) runs. The attention itself is
            # software-pipelined at ki-chunk granularity: chunk i+1's score
            # matmuls interleave with chunk i's ctx matmuls.
            with (
                tc.tile_pool(name="xa", bufs=9) as xpool,
                tc.tile_pool(name="accp", bufs=2, space="PSUM") as accp,
                tc.tile_pool(name="slabp", bufs=19) as slabp,
                tc.tile_pool(name="smallp", bufs=2) as smallp,
                tc.tile_pool(name="coutp", bufs=2) as coutp,
                tc.tile_pool(name="pss", bufs=2, space="PSUM") as pss,
                tc.tile_pool(name="psc", bufs=1, space="PSUM") as psc,
            ):

                def proj_chunk(b_, o):
                    """Project one 512-token slice of batch b_ into qt/kt/vt.

                    Three sequential PSUM accumulation chains over a shared
                    single-slot pool tag keep the PSUM footprint at 2 banks.
                    """
                    tiles = []
                    for kc in range(KD):
                        xt_tile = xpool.tile([P, 512], F32R, name="xt_tile")
                        nc.sync.dma_start(
                            xt_tile[:],
                            xt[kc * P:(kc + 1) * P,
                               b_ * S + o * 512:b_ * S + (o + 1) * 512])
                        tiles.append(xt_tile)
                    for w_sb, dst in ((wq_sb, qt_sb[b_]), (wk_sb, kt_sb[b_]),
                                      (wv_sb, vt_sb[b_])):
                        acc = accp.tile([P, 512], F32, name="acc_ps")
                        for kc in range(KD):
                            _mm(nc, acc[:], w_sb[:, kc], tiles[kc][:],
                                start=(kc == 0), stop=(kc == KD - 1))
                        nc.vector.tensor_copy(
                            dst[:, o * 512:(o + 1) * 512], acc[:])

                def transp_chunk(b_, kc2):
                    """PE-transpose one [64,128] V^T block per head into V'."""
                    for hh in range(2):
                        tp = accp.tile([P, DH], F32R, name="acc_ps")
                        nc.tensor.transpose(
                            tp[:, :DH],
                            vt_sb[b_][hh * DH:(hh + 1) * DH,
                                      kc2 * P:(kc2 + 1) * P],
                            id_sb[hh * DH:(hh + 1) * DH, :])
                        nc.vector.tensor_copy(
                            vp_sb[b_][:, kc2,
                                      hh * (DH + 1):hh * (DH + 1) + DH],
                            tp[:, :DH])
                chunks = [(b_, o) for b_ in range(B) for o in range(S // 512)]
                state = {}

                def emit_scores(idx, kc):
                    b_, o = chunks[idx]
                    qs = slice(o * 512, (o + 1) * 512)
                    ks = slice(kc * P, (kc + 1) * P)
                    s_ps = pss.tile([P, 1024], F32, name="s_ps")
                    _mm(nc, s_ps[:, 0:512], kt_sb[b_][0:DH, ks],
                        qt_sb[b_][0:DH, qs], start=True, stop=True)
                    _mm(nc, s_ps[:, 512:1024], kt_sb[b_][DH:2 * DH, ks],
                        qt_sb[b_][DH:2 * DH, qs], start=True, stop=True)
                    slab = slabp.tile([P, 1024], F32R, name="slab")
                    nc.scalar.activation(slab[:], s_ps[:], AF.Exp, scale=0.125)
                    nc.vector.tensor_scalar_max(slab[:], slab[:], 1.0)
                    state[idx]["slabs"].append(slab)

                def emit_ctx(idx, kc):
                    b_, o = chunks[idx]
                    st_, sp_ = kc == 0, kc == KI - 1
                    c0, c1 = state[idx]["c0"], state[idx]["c1"]
                    slab = state[idx]["slabs"][kc]
                    _mm(nc, c0[:], vp_sb[b_][:, kc, 0:DH + 1], slab[:, 0:512],
                        start=st_, stop=sp_)
                    _mm(nc, c1[:], vp_sb[b_][:, kc, DH + 1:2 * DH + 2],
                        slab[:, 512:1024], start=st_, stop=sp_)

                def emit_normalize(idx):
                    b_, o = chunks[idx]
                    qs = slice(b_ * S + o * 512, b_ * S + (o + 1) * 512)
                    c0, c1 = state[idx]["c0"], state[idx]["c1"]
                    inv0 = smallp.tile([1, 512], F32, name="inv0")
                    inv1 = smallp.tile([1, 512], F32, name="inv1")
                    nc.vector.reciprocal(inv0[:], c0[DH:DH + 1, :])
                    nc.vector.reciprocal(inv1[:], c1[DH:DH + 1, :])
                    inv0b = smallp.tile([DH, 512], F32, name="inv0b")
                    inv1b = smallp.tile([DH, 512], F32, name="inv1b")
                    nc.gpsimd.partition_broadcast(inv0b[:], inv0[:])
                    nc.gpsimd.partition_broadcast(inv1b[:], inv1[:])
                    cout0 = coutp.tile([DH, 512], F32, name="cout0")
                    cout1 = coutp.tile([DH, 512], F32, name="cout1")
                    nc.vector.tensor_mul(cout0[:], c0[0:DH, :], inv0b[:])
                    nc.vector.tensor_mul(cout1[:], c1[0:DH, :], inv1b[:])
                    nc.sync.dma_start(ctx_t[0:DH, qs], cout0[:])
                    nc.sync.dma_start(ctx_t[DH:2 * DH, qs], cout1[:])
                    del state[idx]

                NO = S // 512   # 4 proj chunks per batch
                TPO = KI // NO  # 4 transposes per proj chunk
                # prologue: batch-0 projections with chunk-0 score matmuls
                # folded in per o-slice, so the ScalarE exp pass starts after
                # the first projection chunk (~7 us) instead of after all of
                # batch 0 (~28 us).
                state[0] = {
                    "c0": psc.tile([DH + 1, 512], F32, name="c0"),
                    "c1": psc.tile([DH + 1, 512], F32, name="c1"),
                    "slabs": [],
                }
                for o in range(NO):
                    proj_chunk(0, o)
                    for t in range(TPO):
                        transp_chunk(0, o * TPO + t)
                    for kc in range(o * TPO, (o + 1) * TPO):
                        emit_scores(0, kc)
                # attention, with batch-1 projections/transposes interleaved
                # into batch-0's chunks (idx 1..4)
                for idx in range(1, len(chunks)):
                    state[idx] = {
                        "c0": psc.tile([DH + 1, 512], F32, name="c0"),
                        "c1": psc.tile([DH + 1, 512], F32, name="c1"),
                        "slabs": [],
                    }
                    if idx <= NO:
                        proj_chunk(1, idx - 1)
                        for t in range(TPO):
                            transp_chunk(1, (idx - 1) * TPO + t)
                    for kc in range(KI):
                        emit_scores(idx, kc)
                        emit_ctx(idx - 1, kc)
                    emit_normalize(idx - 1)
                last = len(chunks) - 1
                for kc in range(KI):
                    emit_ctx(last, kc)
                emit_normalize(last)
    nc.compile()
    return nc


# --------------------------------------------------------------------------
# Phase B: per-core row-parallel Wo-proj + AddNorm1 + FFN + AddNorm2.
# Inputs (per core, qi = this core's 512 token rows):
#   ct  [D, QC]    ctx^T slice
#   xts [D, QC]    X^T slice (residual 1)
#   wot [D, D]     Wo^T
#   w1t [D, FF]    W1^T
#   w2t [FF, D]    W2^T
#   g1,be1,g2,be2 [P, D//P]  ln params, feature-on-partition layout
#   b1t [P, FF//P], b2t [P, D//P]
# Output: out_t [D, QC]
# --------------------------------------------------------------------------
def _build_phase_b():
    nc = bacc.Bacc("TRN2", target_bir_lowering=False, debug=False,
                   num_devices=N_CORES)
    ct = nc.dram_tensor("ct", [D, QC], F32R, kind="ExternalInput")
    xts = nc.dram_tensor("xts", [D, QC], F32, kind="ExternalInput")
    wot = nc.dram_tensor("wot", [D, D], F32R, kind="ExternalInput")
    w1t = nc.dram_tensor("w1t", [D, FF], F32R, kind="ExternalInput")
    w2t = nc.dram_tensor("w2t", [FF, D], F32R, kind="ExternalInput")
    g1 = nc.dram_tensor("g1", [P, D // P], F32, kind="ExternalInput")
    be1 = nc.dram_tensor("be1", [P, D // P], F32, kind="ExternalInput")
    g2 = nc.dram_tensor("g2", [P, D // P], F32, kind="ExternalInput")
    be2 = nc.dram_tensor("be2", [P, D // P], F32, kind="ExternalInput")
    b1t = nc.dram_tensor("b1t", [P, FF // P], F32, kind="ExternalInput")
    b2t = nc.dram_tensor("b2t", [P, D // P], F32, kind="ExternalInput")
    out_t = nc.dram_tensor("out_t", [D, QC], F32, kind="ExternalOutput")

    KD = D // P     # 8
    KF = FF // P    # 32

    def layernorm(nc, tc, pools, y_sb, g_sb, be_sb, z_sb, ones, tag):
        """t-layout layernorm: y_sb [P, KD, 512] -> z_sb (may alias layout)."""
        smallp, sqp, bcp = pools
        import contextlib
        ctx = contextlib.ExitStack()
        psst = ctx.enter_context(
            tc.tile_pool(name=f"psst_{tag}", bufs=1, space="PSUM"))
        st_ps = psst.tile([1, 1024], F32, name="st_ps")
        for kc in range(KD):
            _mm(nc, st_ps[:, 0:512], ones[:], y_sb[:, kc],
                             start=(kc == 0), stop=(kc == KD - 1))
        for kc in range(KD):
            sq = sqp.tile([P, 512], F32R, name="sq")
            nc.vector.tensor_mul(sq[:], y_sb[:, kc], y_sb[:, kc])
            _mm(nc, st_ps[:, 512:1024], ones[:], sq[:],
                             start=(kc == 0), stop=(kc == KD - 1))
        stats = smallp.tile([1, 1024], F32, name="stats")
        nc.vector.tensor_scalar(out=stats[:], in0=st_ps[:], scalar1=1.0 / D,
                                scalar2=None, op0=OP.mult)
        mean = stats[:, 0:512]
        ex2 = stats[:, 512:1024]
        msq = smallp.tile([1, 512], F32, name="msq")
        nc.vector.tensor_mul(msq[:], mean, mean)
        var = smallp.tile([1, 512], F32, name="var")
        nc.vector.tensor_sub(var[:], ex2, msq[:])
        nc.vector.tensor_scalar_add(var[:], var[:], EPS)
        std = smallp.tile([1, 512], F32, name="std")
        nc.scalar.activation(std[:], var[:], AF.Sqrt)
        rstd = smallp.tile([1, 512], F32, name="rstd")
        nc.vector.reciprocal(rstd[:], std[:])
        ms = smallp.tile([1, 512], F32, name="ms")
        nc.vector.tensor_mul(ms[:], mean, rstd[:])
        rstd_b = bcp.tile([P, 512], F32, name="rstd_b")
        ms_b = bcp.tile([P, 512], F32, name="ms_b")
        nc.gpsimd.partition_broadcast(rstd_b[:], rstd[:])
        nc.gpsimd.partition_broadcast(ms_b[:], ms[:])
        for kc in range(KD):
            t = sqp.tile([P, 512], F32, name="t_ln")
            nc.vector.tensor_mul(t[:], y_sb[:, kc], rstd_b[:])
            nc.vector.tensor_sub(t[:], t[:], ms_b[:])
            nc.vector.tensor_scalar(out=z_sb[:, kc], in0=t[:],
                                    scalar1=g_sb[:, kc:kc + 1],
                                    scalar2=be_sb[:, kc:kc + 1],
                                    op0=OP.mult, op1=OP.add)
        ctx.close()

    with tile.TileContext(nc) as tc:
        with (
            tc.tile_pool(name="persist", bufs=1) as persist,
            tc.tile_pool(name="wp", bufs=6) as wp,
            tc.tile_pool(name="sqp", bufs=3) as sqp,
            tc.tile_pool(name="smallp", bufs=1) as smallp,
            tc.tile_pool(name="bcp", bufs=2) as bcp,
        ):
            ct_sb = persist.tile([P, KD, QC], F32R)
            xts_sb = persist.tile([P, KD, QC], F32)
            y1_sb = persist.tile([P, KD, QC], F32R)
            z1_sb = persist.tile([P, KD, QC], F32R)
            h_sb = persist.tile([P, KF, QC], F32R)
            # y2 reuses y1's slot (y1 dead after LN1); z2 reuses ct's (dead
            # after B1). Tag sharing makes Tile serialize via WAR edges.
            y2_sb = persist.tile([P, KD, QC], F32R, tag="y1_sb")
            z2_sb = persist.tile([P, KD, QC], F32, tag="ct_sb")
            g1_sb = persist.tile([P, KD], F32)
            be1_sb = persist.tile([P, KD], F32)
            g2_sb = persist.tile([P, KD], F32)
            be2_sb = persist.tile([P, KD], F32)
            b1t_sb = persist.tile([P, KF], F32)
            b2t_sb = persist.tile([P, KD], F32)
            ones = persist.tile([P, 1], F32R)

            nc.sync.dma_start(ct_sb[:], ct.ap().rearrange("(kc p) q -> p kc q", p=P))
            nc.sync.dma_start(xts_sb[:], xts.ap().rearrange("(kc p) q -> p kc q", p=P))
            for t_sb, t_dr in ((g1_sb, g1), (be1_sb, be1), (g2_sb, g2),
                               (be2_sb, be2), (b1t_sb, b1t), (b2t_sb, b2t)):
                nc.sync.dma_start(t_sb[:], t_dr.ap())
            nc.vector.memset(ones[:].bitcast(F32), 1.0)

            # ---- B1: att_out = Wo @ ct (+ X residual) ----
            with tc.tile_pool(name="psa", bufs=1, space="PSUM") as psa:
                for mg in range(2):
                    a_ps = [psa.tile([P, 512], F32, name=f"mm_ps{i}")
                            for i in range(4)]
                    for kc in range(KD):
                        w_tile = wp.tile([P, 512], F32R, name="wo_tile")
                        nc.sync.dma_start(
                            w_tile[:],
                            wot[kc * P:(kc + 1) * P, mg * 512:(mg + 1) * 512])
                        for i in range(4):
                            _mm(nc, a_ps[i][:],
                                w_tile[:, i * P:(i + 1) * P], ct_sb[:, kc],
                                start=(kc == 0), stop=(kc == KD - 1))
                    for i in range(4):
                        m = mg * 4 + i
                        nc.vector.tensor_add(y1_sb[:, m], a_ps[i][:], xts_sb[:, m])

                # ---- LN1 ----
                layernorm(nc, tc, (smallp, sqp, bcp), y1_sb, g1_sb, be1_sb,
                          z1_sb, ones, "ln1")

            # ---- FFN1 + FFN2: the first output half of FFN2 (mg0) is
            # interleaved into the FFN1 loop so W2 @ h starts consuming h
            # chunks as soon as they exist; tile sizes stay [128,512] so the
            # DMA instruction count is unchanged. 8 PSUM banks: 4 h + 4 f. ----
            with tc.tile_pool(name="psa2", bufs=1, space="PSUM") as psa2:
                f_ps = [psa2.tile([P, 512], F32, name=f"f_ps{i}")
                        for i in range(4)]
                for fg in range(KF // 4):
                    h_ps = [psa2.tile([P, 512], F32, name=f"h_ps{i}")
                            for i in range(4)]
                    for kc in range(KD):
                        w_tile = wp.tile([P, 512], F32R, name="w1_tile")
                        nc.sync.dma_start(
                            w_tile[:],
                            w1t[kc * P:(kc + 1) * P, fg * 512:(fg + 1) * 512])
                        for i in range(4):
                            _mm(nc, h_ps[i][:],
                                w_tile[:, i * P:(i + 1) * P], z1_sb[:, kc],
                                start=(kc == 0), stop=(kc == KD - 1))
                    for i in range(4):
                        fm = fg * 4 + i
                        nc.scalar.activation(h_sb[:, fm], h_ps[i][:], AF.Relu,
                                             bias=b1t_sb[:, fm:fm + 1])
                    for i in range(4):
                        fk = fg * 4 + i
                        w_tile = wp.tile([P, 512], F32R, name="w2_tile")
                        nc.sync.dma_start(
                            w_tile[:], w2t[fk * P:(fk + 1) * P, 0:512])
                        for j in range(4):
                            _mm(nc, f_ps[j][:],
                                w_tile[:, j * P:(j + 1) * P], h_sb[:, fk],
                                start=(fk == 0), stop=(fk == KF - 1))
                for j in range(4):
                    nc.vector.scalar_tensor_tensor(
                        out=y2_sb[:, j], in0=f_ps[j][:],
                        scalar=b2t_sb[:, j:j + 1], in1=z1_sb[:, j],
                        op0=OP.add, op1=OP.add)

                f_ps2 = [psa2.tile([P, 512], F32, name=f"f_ps{i}")
                         for i in range(4)]
                for fk in range(KF):
                    w_tile = wp.tile([P, 512], F32R, name="w2_tile")
                    nc.sync.dma_start(
                        w_tile[:], w2t[fk * P:(fk + 1) * P, 512:1024])
                    for j in range(4):
                        _mm(nc, f_ps2[j][:],
                            w_tile[:, j * P:(j + 1) * P], h_sb[:, fk],
                            start=(fk == 0), stop=(fk == KF - 1))
                for j in range(4):
                    m = 4 + j
                    nc.vector.scalar_tensor_tensor(
                        out=y2_sb[:, m], in0=f_ps2[j][:],
                        scalar=b2t_sb[:, m:m + 1], in1=z1_sb[:, m],
                        op0=OP.add, op1=OP.add)

            # ---- LN2 ----
            layernorm(nc, tc, (smallp, sqp, bcp), y2_sb, g2_sb, be2_sb,
                      z2_sb, ones, "ln2")

            for kc in range(KD):
                nc.sync.dma_start(out_t[kc * P:(kc + 1) * P, :], z2_sb[:, kc])
    nc.compile()
    return nc


def _get(name, builder):
    if name not in _CACHE:
        _CACHE[name] = builder()
    return _CACHE[name]


def _prep_inputs(X, Wq, Wk, Wo, ln1_g, ln1_b, ln2_g, ln2_b, W1, b1, W2, b2):
    """Host-side sharding/layout. Returns (in_maps_a, in_maps_b_builder, Xt)."""
    f = lambda a: np.ascontiguousarray(np.asarray(a, dtype=np.float32))
    Xt = f(np.asarray(X, np.float32).reshape(N, D).T)        # [D, N]
    WqT, WkT, WoT = f(np.asarray(Wq).T), f(np.asarray(Wk).T), f(np.asarray(Wo).T)
    W1T, W2T = f(np.asarray(W1).T), f(np.asarray(W2).T)      # [D,FF], [FF,D]
    vecP = lambda v, k: f(np.asarray(v).reshape(k, P).T)     # [P, k]
    g1v, be1v = vecP(ln1_g, D // P), vecP(ln1_b, D // P)
    g2v, be2v = vecP(ln2_g, D // P), vecP(ln2_b, D // P)
    b1v, b2v = vecP(b1, FF // P), vecP(b2, D // P)

    idm = np.tile(np.eye(DH, dtype=np.float32), (2, 1))   # [128, 64]
    in_maps_a = [
        {
            "xt": Xt,
            "idm": idm,
            "wqt": f(WqT[:, c * P:(c + 1) * P]),
            "wkt": f(WkT[:, c * P:(c + 1) * P]),
            "wvt": f(WoT[:, c * P:(c + 1) * P]),
        }
        for c in range(N_CORES)
    ]

    def in_maps_b(ct_full):
        return [
            {
                "ct": f(ct_full[:, c * QC:(c + 1) * QC]),
                "xts": f(Xt[:, c * QC:(c + 1) * QC]),
                "wot": WoT, "w1t": W1T, "w2t": W2T,
                "g1": g1v, "be1": be1v, "g2": g2v, "be2": be2v,
                "b1t": b1v, "b2t": b2v,
            }
            for c in range(N_CORES)
        ]

    return in_maps_a, in_maps_b


def kernel(X, Wq, Wk, Wo, ln1_g, ln1_b, ln2_g, ln2_b, W1, b1, W2, b2):
    in_maps_a, in_maps_b = _prep_inputs(
        X, Wq, Wk, Wo, ln1_g, ln1_b, ln2_g, ln2_b, W1, b1, W2, b2)

    nc_a = _get("a", _build_phase_a)
    res_a = run_bass_kernel_spmd(nc_a, in_maps_a, core_ids=list(range(N_CORES)))
    ct_full = np.concatenate(
        [res_a.results[c]["ctx_t"] for c in range(N_CORES)], axis=0)  # [D, N]

    nc_b = _get("b", _build_phase_b)
    res_b = run_bass_kernel_spmd(nc_b, in_maps_b(ct_full),
                                 core_ids=list(range(N_CORES)))
    out_t = np.concatenate(
        [res_b.results[c]["out_t"] for c in range(N_CORES)], axis=1)  # [D, N]
    return np.ascontiguousarray(out_t.T).reshape(B, S, D).astype(np.float32)
